# revision 35
# baseline (speedup 1.0000x reference)
"""Trainium2 Bass kernel for nn_CriticModel (BSDE critic-delta forward).

Math (see reference):
  xt = all (sample, t<64) points of x           [B*64, 256]
  u = actor_mlp(xt); w = |xt|^2 + |u|^2         (BN folded into dense layers)
  y = sum_t w * weight(t; tau, ExitIndex)
  delta = y + where(ExitIndex<64, |x_tau|^2, critic(x_tau)) - critic(x[:,:,0])

Strategy: pure data parallel over 8 NeuronCores (512 samples each).
On-device per core: stream groups of 8 samples x 64 timesteps (512 points,
features on partitions, 2 K-chunks of 128), run the 3-layer folded MLP on
the PE, square+ones-matmul reductions for |x|^2 and |u|^2, then a
per-sample weighted time reduction. Critic runs on x_tau (PE-transposed)
and on gathered t=0 columns. BN folding, the ragged time-weight matrix and
the exit mask are precomputed on host (tiny tensors only).

Dtype modes (BASS_MM_MODE):
  f32r (default): fp32r weights + activations — fp32 bits stream through the
        PE at 1 cycle/col (N>=256) with no cast passes and ~1e-4 accuracy.
  bf16: everything bf16 (DVE cast passes for x).
  f32:  plain fp32 matmuls (4 cycles/col, slow; reference fallback).
  (mix bf16xf32r is rejected by walrus: no 32-bit x 16-bit matmuls.)
"""

import os
import sys
from contextlib import ExitStack

for _p in ("/opt/trn_rl_repo",):
    if _p not in sys.path:
        sys.path.insert(0, _p)

import numpy as np

import concourse.bass as bass
import concourse.tile as tile
from concourse import bacc, mybir
from concourse.bass_utils import run_bass_kernel_spmd
from concourse.masks import make_identity

DIM = 256
N_T = 64
DT = 1.0 / N_T
BN_EPS = 1e-6
B = 4096
NCORES = 8
BL = B // NCORES          # samples per core
NB = 8                    # samples per group (NB*N_T = 512 matmul columns)
NG = BL // NB             # groups per core
NPTS = NB * N_T           # 512
F32 = mybir.dt.float32
BF16 = mybir.dt.bfloat16
F32R = mybir.dt.float32r

MM_MODE = os.environ.get("BASS_MM_MODE", "f32r")


def _w_dt():
    """Weight (stationary operand) dtype."""
    return {"mix": BF16, "bf16": BF16, "f32r": F32R, "f32": F32}[MM_MODE]


def _a_dt():
    """Activation (moving operand) storage dtype."""
    return {"mix": F32R, "bf16": BF16, "f32r": F32R, "f32": F32}[MM_MODE]


def _mm_np(a):
    """Host-side storage format for weights."""
    if _w_dt() == BF16:
        import ml_dtypes
        return np.asarray(a, np.float32).astype(ml_dtypes.bfloat16)
    return np.asarray(a, np.float32)


def _kernel_body(ctx, tc):
    nc = tc.nc
    wdt = _w_dt()
    adt = _a_dt()

    # x arrives host-resharded as per-(group, k-chunk) contiguous blocks so
    # each DMA descriptor moves a full 2KB partition row (vs 256B strided).
    xdt = adt if MM_MODE == "f32r" else F32
    x_d = nc.dram_tensor("x", [NG, 2, 128, NB, N_T], xdt, kind="ExternalInput").ap()
    xtau_d = nc.dram_tensor("x_tau", [BL, DIM], F32, kind="ExternalInput").ap()
    wmat_d = nc.dram_tensor("wmat", [NG, 1, NPTS], F32, kind="ExternalInput").ap()
    mask_d = nc.dram_tensor("mask", [1, BL], F32, kind="ExternalInput").ap()
    crow_d = nc.dram_tensor("crow", [1, BL], F32, kind="ExternalInput").ap()
    aw_d = [nc.dram_tensor(f"aw{i}", [DIM, DIM], wdt, kind="ExternalInput").ap()
            for i in range(3)]
    ab_d = [nc.dram_tensor(f"ab{i}", [DIM], F32, kind="ExternalInput").ap()
            for i in range(3)]
    cw_d = [nc.dram_tensor(f"cw{i}", [DIM, DIM], wdt, kind="ExternalInput").ap()
            for i in range(2)]
    cb_d = [nc.dram_tensor(f"cb{i}", [DIM], F32, kind="ExternalInput").ap()
            for i in range(2)]
    cv_d = nc.dram_tensor("cv", [DIM], wdt, kind="ExternalInput").ap()
    ones_d = nc.dram_tensor("ones", [128], wdt, kind="ExternalInput").ap()
    out_d = nc.dram_tensor("delta", [1, BL], F32, kind="ExternalOutput").ap()

    consts = ctx.enter_context(tc.tile_pool(name="consts", bufs=1))
    xin = ctx.enter_context(tc.tile_pool(name="xin", bufs=6))
    acts = ctx.enter_context(tc.tile_pool(name="acts", bufs=6))
    rows = ctx.enter_context(tc.tile_pool(name="rows", bufs=4))
    persist = ctx.enter_context(tc.tile_pool(name="persist", bufs=1))
    psum = ctx.enter_context(tc.tile_pool(name="psum", bufs=5, space="PSUM"))
    wrow = ctx.enter_context(tc.tile_pool(name="wrow", bufs=3, space="PSUM"))

    # ---- constants ----
    # weight tiles: [K=128, kc, M=256]; lhsT slice = [:, kc, m*128:(m+1)*128]
    def load_w(dram, nm):
        t = consts.tile([128, 2, DIM], wdt, tag=nm, name=nm)
        for kc in range(2):
            nc.sync.dma_start(out=t[:, kc, :], in_=dram[kc * 128:(kc + 1) * 128, :])
        return t

    aw = [load_w(d, f"aw{i}_t") for i, d in enumerate(aw_d)]
    cw = [load_w(d, f"cw{i}_t") for i, d in enumerate(cw_d)]

    def load_b(dram, nm):
        t = consts.tile([128, 2], F32, tag=nm, name=nm)
        nc.sync.dma_start(out=t, in_=dram.rearrange("(m p) -> p m", p=128))
        return t

    ab = [load_b(d, f"ab{i}_t") for i, d in enumerate(ab_d)]
    cb = [load_b(d, f"cb{i}_t") for i, d in enumerate(cb_d)]
    cv = consts.tile([128, 2], wdt, tag="cv")
    nc.sync.dma_start(out=cv, in_=cv_d.rearrange("(m p) -> p m", p=128))

    ones_t = consts.tile([128, 1], wdt, tag="ones")
    nc.sync.dma_start(out=ones_t, in_=ones_d.rearrange("(p o) -> p o", o=1))
    ident = consts.tile([128, 128], F32, tag="ident")
    make_identity(nc, ident)
    mask_t = consts.tile([1, BL], F32, tag="mask")
    nc.sync.dma_start(out=mask_t, in_=mask_d)
    crow_t = consts.tile([1, BL], F32, tag="crow")
    nc.sync.dma_start(out=crow_t, in_=crow_d)

    # ---- persistent ----
    x0b = [persist.tile([128, BL], adt, tag=f"x0b{k}", name=f"x0b{k}")
           for k in range(2)]
    xtauT = [persist.tile([128, BL], adt, tag=f"xtauT{k}", name=f"xtauT{k}")
             for k in range(2)]
    y_buf = persist.tile([1, BL], F32, tag="y_buf")
    z_row = persist.tile([1, BL], F32, tag="z_row")
    vtau_row = persist.tile([1, BL], F32, tag="vtau_row")
    v0_row = persist.tile([1, BL], F32, tag="v0_row")

    def critic(chunks, out_row):
        """3-layer folded critic on [128, BL] feature-major chunks -> [1, BL]."""
        h = chunks
        for layer in range(2):
            nh = []
            for m in range(2):
                ps = psum.tile([128, BL], F32, tag="mm")
                for kc in range(2):
                    nc.tensor.matmul(ps, cw[layer][:, kc, m * 128:(m + 1) * 128],
                                     h[kc], start=(kc == 0), stop=(kc == 1))
                ht = acts.tile([128, BL], adt, tag="h")
                nc.scalar.activation(out=ht, in_=ps,
                                     func=mybir.ActivationFunctionType.Relu,
                                     bias=cb[layer][:, m:m + 1], scale=1.0)
                nh.append(ht)
            h = nh
        vp = wrow.tile([1, BL], F32, tag="w")
        for kc in range(2):
            nc.tensor.matmul(vp, cv[:, kc:kc + 1], h[kc],
                             start=(kc == 0), stop=(kc == 1))
        nc.vector.tensor_copy(out=out_row, in_=vp)

    # ---- x_tau: natural load + PE transpose into feature-major chunks ----
    for pt in range(BL // 128):
        nat = xin.tile([128, DIM], F32, tag="xtau_nat", name=f"nat{pt}")
        nc.sync.dma_start(out=nat, in_=xtau_d[pt * 128:(pt + 1) * 128, :])
        for kc in range(2):
            ps = psum.tile([128, 128], F32, tag="mm", name=f"tr{kc}")
            nc.tensor.transpose(ps, nat[:, kc * 128:(kc + 1) * 128], ident)
            nc.scalar.copy(out=xtauT[kc][:, pt * 128:(pt + 1) * 128], in_=ps)

    # Z = |x_tau|^2 via square + ones-matmul
    zp = wrow.tile([1, BL], F32, tag="w", name="zp")
    for kc in range(2):
        sq = acts.tile([128, BL], adt, tag="sq", name=f"zsq{kc}")
        nc.scalar.square(out=sq, in_=xtauT[kc])
        nc.tensor.matmul(zp, ones_t, sq, start=(kc == 0), stop=(kc == 1))
    nc.vector.tensor_copy(out=z_row, in_=zp)

    critic(xtauT, vtau_row)

    # ---- main loop: actor MLP + squared-norm reduction over all points ----
    for g in range(NG):
        xt = []
        for kc in range(2):
            t = xin.tile([128, NB, N_T], xdt, tag="x")
            nc.sync.dma_start(out=t, in_=x_d[g, kc])
            xt.append(t)
        xr = [t.rearrange("p b t -> p (b t)") for t in xt]

        if MM_MODE == "bf16":
            xb = []
            for kc in range(2):
                c = acts.tile([128, NPTS], BF16, tag="xb", name=f"xb{kc}")
                nc.vector.tensor_copy(out=c, in_=xr[kc])
                xb.append(c)
        else:
            xb = xr

        # x0 gather (t=0 column of each sample) for the end-of-kernel critic
        for kc in range(2):
            nc.gpsimd.tensor_copy(
                out=x0b[kc][:, g * NB:(g + 1) * NB],
                in_=xt[kc][:, :, 0:1].rearrange("p b t -> p (b t)"))

        # layer 1 (relu on ACT), layer 2 (relu on DVE), layer 3 -> u^2 on ACT
        h = xb
        for layer in range(2):
            nh = []
            for m in range(2):
                ps = psum.tile([128, NPTS], F32, tag="mm", name=f"ps{layer}{m}")
                for kc in range(2):
                    nc.tensor.matmul(ps, aw[layer][:, kc, m * 128:(m + 1) * 128],
                                     h[kc], start=(kc == 0), stop=(kc == 1))
                ht = acts.tile([128, NPTS], adt, tag="h", name=f"h{layer}{m}")
                if layer == 0:
                    nc.scalar.activation(out=ht, in_=ps,
                                         func=mybir.ActivationFunctionType.Relu,
                                         bias=ab[layer][:, m:m + 1], scale=1.0)
                else:
                    nc.vector.tensor_scalar(
                        out=ht, in0=ps, scalar1=ab[layer][:, m:m + 1], scalar2=0.0,
                        op0=mybir.AluOpType.add, op1=mybir.AluOpType.max)
                nh.append(ht)
            h = nh

        wp = wrow.tile([1, NPTS], F32, tag="w", name="wp")
        u2s = []
        for m in range(2):
            ps = psum.tile([128, NPTS], F32, tag="mm", name=f"ps3{m}")
            for kc in range(2):
                nc.tensor.matmul(ps, aw[2][:, kc, m * 128:(m + 1) * 128],
                                 h[kc], start=(kc == 0), stop=(kc == 1))
            u2 = acts.tile([128, NPTS], adt, tag="sq", name=f"u2{m}")
            nc.scalar.activation(out=u2, in_=ps,
                                 func=mybir.ActivationFunctionType.Square,
                                 bias=ab[2][:, m:m + 1], scale=1.0)
            u2s.append(u2)
        # sum the x^2 chunk into the u^2 chunk elementwise, then reduce once:
        # halves the ones-matmul count (2 PE streams/group instead of 4)
        for kc in range(2):
            sq = acts.tile([128, NPTS], adt, tag="sq", name=f"sq{kc}")
            nc.gpsimd.tensor_mul(sq, xb[kc], xb[kc])
            s = acts.tile([128, NPTS], adt, tag="st", name=f"s{kc}", bufs=4)
            eng = nc.vector if kc == 0 else nc.gpsimd
            eng.tensor_add(s, sq, u2s[kc])
            nc.tensor.matmul(wp, ones_t, s, start=(kc == 0), stop=(kc == 1))

        # weighted per-sample time reduction
        wm = rows.tile([1, NPTS], F32, tag="wm")
        nc.sync.dma_start(out=wm, in_=wmat_d[g])
        ws = rows.tile([1, NPTS], F32, tag="ws")
        nc.vector.tensor_mul(ws, wp, wm)
        nc.vector.reduce_sum(out=y_buf[0:1, g * NB:(g + 1) * NB],
                             in_=ws.rearrange("p (b t) -> p b t", b=NB),
                             axis=mybir.AxisListType.X)

    critic(x0b, v0_row)

    # ---- final combine: delta = y + mask*(Z - vtau) + vtau - v0 + crow ----
    f = [rows.tile([1, BL], F32, tag="fin", name=f"fin{_i}") for _i in range(2)]
    nc.vector.tensor_sub(f[0], z_row, vtau_row)
    nc.vector.tensor_mul(f[1], mask_t, f[0])
    nc.vector.tensor_add(f[0], f[1], vtau_row)
    nc.vector.tensor_sub(f[1], f[0], v0_row)
    nc.vector.tensor_add(f[0], f[1], y_buf)
    out_t = rows.tile([1, BL], F32, tag="out")
    nc.vector.tensor_add(out_t, f[0], crow_t)
    nc.sync.dma_start(out=out_d, in_=out_t)


if os.environ.get("BASS_LDW_OPT", "0") == "1":
    # experiment: let walrus pipeline/dedup LDWEIGHTS (off by default upstream)
    import concourse.bass_utils as _BU
    _orig_run_command = _BU.run_command

    def _run_command_ldwopt(argv, **kwargs):
        argv = ["--enable-ldw-opt=true" if a == "--enable-ldw-opt=false" else a
                for a in argv]
        return _orig_run_command(argv, **kwargs)

    _BU.run_command = _run_command_ldwopt


_BUILT = None


def build():
    global _BUILT
    if _BUILT is not None:
        return _BUILT
    nc = bacc.Bacc("TRN2", target_bir_lowering=False, debug=False)
    with tile.TileContext(nc) as tc:
        with ExitStack() as ctx:
            _kernel_body(ctx, tc)
    nc.compile()
    _BUILT = nc
    return nc


def _affine(bn):
    g, b, m, v = (np.asarray(t, np.float64) for t in bn)
    a = g / np.sqrt(v + BN_EPS)
    return a, b - m * a


def fold_params(p):
    a_in, c_in = _affine(p['bn_in'])
    W1, W2 = (np.asarray(W, np.float64) for W in p['Ws'])
    (a1, c1), (a2, c2) = (_affine(bn) for bn in p['bns'])
    Wout = np.asarray(p['Wout'], np.float64)
    bout = np.asarray(p['bout'], np.float64)
    a_o, c_o = _affine(p['bn_out'])
    U1 = (a_in[:, None] * W1) * a1[None, :]
    d1 = (c_in @ W1) * a1 + c1
    U2 = W2 * a2[None, :]
    U3 = Wout * a_o[None, :]
    d3 = bout * a_o + c_o
    return [(U1, d1), (U2, c2), (U3, d3)]


def prepare_in_maps(x, x_tau, tau, ExitIndex, critic_params, actor_params):
    x = np.asarray(x, np.float32)
    # reshard + relayout: [B, 256, 65] -> per-core [NG, 2, 128, NB, N_T] blocks
    xw = x[:, :, :N_T].reshape(NCORES, NG, NB, 2, 128, N_T)
    xc = np.ascontiguousarray(xw.transpose(0, 1, 3, 4, 2, 5))
    x_tau = np.ascontiguousarray(np.asarray(x_tau, np.float32))
    tau = np.asarray(tau, np.float32)
    ei = np.asarray(ExitIndex, np.int32)

    af = fold_params(actor_params)
    cf = fold_params(critic_params)

    t = np.arange(N_T, dtype=np.int32)[None, :]
    eib = ei[:, None]
    wmat = np.where(t < eib, np.float32(DT),
                    np.where(t == eib, tau[:, None] - t.astype(np.float32) * np.float32(DT),
                             np.float32(0))).astype(np.float32)
    maskF = (ei < N_T).astype(np.float32)
    d3c = np.float32(cf[2][1][0])
    crow = (-maskF * d3c).astype(np.float32)

    shared = {
        "aw0": _mm_np(af[0][0]), "aw1": _mm_np(af[1][0]), "aw2": _mm_np(af[2][0]),
        "ab0": np.asarray(af[0][1], np.float32),
        "ab1": np.asarray(af[1][1], np.float32),
        "ab2": np.asarray(af[2][1], np.float32),
        "cw0": _mm_np(cf[0][0]), "cw1": _mm_np(cf[1][0]),
        "cb0": np.asarray(cf[0][1], np.float32),
        "cb1": np.asarray(cf[1][1], np.float32),
        "cv": _mm_np(cf[2][0][:, 0]),
        "ones": _mm_np(np.ones(128)),
    }

    in_maps = []
    for c in range(NCORES):
        s = slice(c * BL, (c + 1) * BL)
        in_maps.append({
            "x": xc[c],
            "x_tau": x_tau[s],
            "wmat": np.ascontiguousarray(wmat[s].reshape(NG, 1, NPTS)),
            "mask": maskF[s].reshape(1, BL),
            "crow": crow[s].reshape(1, BL),
            **shared,
        })
    return in_maps


def kernel(dw, x, x_tau, tau, ExitIndex, critic_params, actor_params):
    in_maps = prepare_in_maps(x, x_tau, tau, ExitIndex, critic_params, actor_params)
    nc = build()
    try:
        res = run_bass_kernel_spmd(nc, in_maps, core_ids=list(range(NCORES)))
    except Exception:
        # transient NRT_EXEC_UNIT_UNRECOVERABLE has been observed to clear on
        # the next attempt; retry once before giving up
        res = run_bass_kernel_spmd(nc, in_maps, core_ids=list(range(NCORES)))
    delta = np.concatenate([res.results[c]["delta"][0] for c in range(NCORES)])
    return delta.astype(np.float32)


if __name__ == "__main__":
    build()
    print("build ok")


# revision 36
# speedup vs baseline: 1.0927x; 1.0927x over previous
"""Trainium2 Bass kernel for nn_CriticModel (BSDE critic-delta forward).

Math (see reference):
  xt = all (sample, t<64) points of x           [B*64, 256]
  u = actor_mlp(xt); w = |xt|^2 + |u|^2         (BN folded into dense layers)
  y = sum_t w * weight(t; tau, ExitIndex)
  delta = y + where(ExitIndex<64, |x_tau|^2, critic(x_tau)) - critic(x[:,:,0])

Strategy: pure data parallel over 8 NeuronCores (512 samples each).
On-device per core: stream groups of 8 samples x 64 timesteps (512 points,
features on partitions, 2 K-chunks of 128), run the 3-layer folded MLP on
the PE, square+ones-matmul reductions for |x|^2 and |u|^2, then a
per-sample weighted time reduction. Critic runs on x_tau (PE-transposed)
and on gathered t=0 columns. BN folding, the ragged time-weight matrix and
the exit mask are precomputed on host (tiny tensors only).

Dtype modes (BASS_MM_MODE):
  f32r (default): fp32r weights + activations — fp32 bits stream through the
        PE at 1 cycle/col (N>=256) with no cast passes and ~1e-4 accuracy.
  bf16: everything bf16 (DVE cast passes for x).
  f32:  plain fp32 matmuls (4 cycles/col, slow; reference fallback).
  (mix bf16xf32r is rejected by walrus: no 32-bit x 16-bit matmuls.)
"""

import os
import sys
from contextlib import ExitStack

for _p in ("/opt/trn_rl_repo",):
    if _p not in sys.path:
        sys.path.insert(0, _p)

import numpy as np

import concourse.bass as bass
import concourse.tile as tile
from concourse import bacc, mybir
from concourse.bass_utils import run_bass_kernel_spmd
from concourse.masks import make_identity

DIM = 256
N_T = 64
DT = 1.0 / N_T
BN_EPS = 1e-6
B = 4096
NCORES = 8
BL = B // NCORES          # samples per core
NB = 8                    # samples per group (NB*N_T = 512 matmul columns)
NG = BL // NB             # groups per core
NPTS = NB * N_T           # 512
F32 = mybir.dt.float32
BF16 = mybir.dt.bfloat16
F32R = mybir.dt.float32r

MM_MODE = os.environ.get("BASS_MM_MODE", "f32r")


def _w_dt():
    """Weight (stationary operand) dtype."""
    return {"mix": BF16, "bf16": BF16, "f32r": F32R, "f32": F32}[MM_MODE]


def _a_dt():
    """Activation (moving operand) storage dtype."""
    return {"mix": F32R, "bf16": BF16, "f32r": F32R, "f32": F32}[MM_MODE]


def _mm_np(a):
    """Host-side storage format for weights."""
    if _w_dt() == BF16:
        import ml_dtypes
        return np.asarray(a, np.float32).astype(ml_dtypes.bfloat16)
    return np.asarray(a, np.float32)


def _kernel_body(ctx, tc):
    nc = tc.nc
    wdt = _w_dt()
    adt = _a_dt()

    # x arrives host-resharded as per-(group, k-chunk) contiguous blocks so
    # each DMA descriptor moves a full 2KB partition row (vs 256B strided).
    xdt = adt if MM_MODE == "f32r" else F32
    x_d = nc.dram_tensor("x", [NG, 2, 128, NB, N_T], xdt, kind="ExternalInput").ap()
    xtau_d = nc.dram_tensor("x_tau", [BL, DIM], F32, kind="ExternalInput").ap()
    wmat_d = nc.dram_tensor("wmat", [NG, 1, NPTS], F32, kind="ExternalInput").ap()
    mask_d = nc.dram_tensor("mask", [1, BL], F32, kind="ExternalInput").ap()
    crow_d = nc.dram_tensor("crow", [1, BL], F32, kind="ExternalInput").ap()
    aw_d = [nc.dram_tensor(f"aw{i}", [DIM, DIM], wdt, kind="ExternalInput").ap()
            for i in range(3)]
    ab_d = [nc.dram_tensor(f"ab{i}", [DIM], F32, kind="ExternalInput").ap()
            for i in range(3)]
    cw_d = [nc.dram_tensor(f"cw{i}", [DIM, DIM], wdt, kind="ExternalInput").ap()
            for i in range(2)]
    cb_d = [nc.dram_tensor(f"cb{i}", [DIM], F32, kind="ExternalInput").ap()
            for i in range(2)]
    cv_d = nc.dram_tensor("cv", [DIM], wdt, kind="ExternalInput").ap()
    ones_d = nc.dram_tensor("ones", [128], wdt, kind="ExternalInput").ap()
    out_d = nc.dram_tensor("delta", [1, BL], F32, kind="ExternalOutput").ap()

    consts = ctx.enter_context(tc.tile_pool(name="consts", bufs=1))
    xin = ctx.enter_context(tc.tile_pool(name="xin", bufs=6))
    acts = ctx.enter_context(tc.tile_pool(name="acts", bufs=6))
    rows = ctx.enter_context(tc.tile_pool(name="rows", bufs=4))
    persist = ctx.enter_context(tc.tile_pool(name="persist", bufs=1))
    psum = ctx.enter_context(tc.tile_pool(name="psum", bufs=5, space="PSUM"))
    wrow = ctx.enter_context(tc.tile_pool(name="wrow", bufs=3, space="PSUM"))

    # ---- constants ----
    # weight tiles: [K=128, kc, M=256]; lhsT slice = [:, kc, m*128:(m+1)*128]
    def load_w(dram, nm):
        t = consts.tile([128, 2, DIM], wdt, tag=nm, name=nm)
        for kc in range(2):
            nc.sync.dma_start(out=t[:, kc, :], in_=dram[kc * 128:(kc + 1) * 128, :])
        return t

    aw = [load_w(d, f"aw{i}_t") for i, d in enumerate(aw_d)]
    cw = [load_w(d, f"cw{i}_t") for i, d in enumerate(cw_d)]

    def load_b(dram, nm):
        t = consts.tile([128, 2], F32, tag=nm, name=nm)
        nc.sync.dma_start(out=t, in_=dram.rearrange("(m p) -> p m", p=128))
        return t

    ab = [load_b(d, f"ab{i}_t") for i, d in enumerate(ab_d)]
    cb = [load_b(d, f"cb{i}_t") for i, d in enumerate(cb_d)]
    cv = consts.tile([128, 2], wdt, tag="cv")
    nc.sync.dma_start(out=cv, in_=cv_d.rearrange("(m p) -> p m", p=128))

    ones_t = consts.tile([128, 1], wdt, tag="ones")
    nc.sync.dma_start(out=ones_t, in_=ones_d.rearrange("(p o) -> p o", o=1))
    ident = consts.tile([128, 128], F32, tag="ident")
    make_identity(nc, ident)
    mask_t = consts.tile([1, BL], F32, tag="mask")
    nc.sync.dma_start(out=mask_t, in_=mask_d)
    crow_t = consts.tile([1, BL], F32, tag="crow")
    nc.sync.dma_start(out=crow_t, in_=crow_d)

    # ---- persistent ----
    x0b = [persist.tile([128, BL], adt, tag=f"x0b{k}", name=f"x0b{k}")
           for k in range(2)]
    xtauT = [persist.tile([128, BL], adt, tag=f"xtauT{k}", name=f"xtauT{k}")
             for k in range(2)]
    y_buf = persist.tile([1, BL], F32, tag="y_buf")
    z_row = persist.tile([1, BL], F32, tag="z_row")
    vtau_row = persist.tile([1, BL], F32, tag="vtau_row")
    v0_row = persist.tile([1, BL], F32, tag="v0_row")

    def critic(chunks, out_row):
        """3-layer folded critic on [128, BL] feature-major chunks -> [1, BL]."""
        h = chunks
        for layer in range(2):
            nh = []
            for m in range(2):
                ps = psum.tile([128, BL], F32, tag="mm")
                for kc in range(2):
                    nc.tensor.matmul(ps, cw[layer][:, kc, m * 128:(m + 1) * 128],
                                     h[kc], start=(kc == 0), stop=(kc == 1))
                ht = acts.tile([128, BL], adt, tag="h")
                nc.scalar.activation(out=ht, in_=ps,
                                     func=mybir.ActivationFunctionType.Relu,
                                     bias=cb[layer][:, m:m + 1], scale=1.0)
                nh.append(ht)
            h = nh
        vp = wrow.tile([1, BL], F32, tag="w")
        for kc in range(2):
            nc.tensor.matmul(vp, cv[:, kc:kc + 1], h[kc],
                             start=(kc == 0), stop=(kc == 1))
        nc.vector.tensor_copy(out=out_row, in_=vp)

    # ---- x_tau: natural load + PE transpose into feature-major chunks ----
    for pt in range(BL // 128):
        nat = xin.tile([128, DIM], F32, tag="xtau_nat", name=f"nat{pt}")
        nc.sync.dma_start(out=nat, in_=xtau_d[pt * 128:(pt + 1) * 128, :])
        for kc in range(2):
            ps = psum.tile([128, 128], F32, tag="mm", name=f"tr{kc}")
            nc.tensor.transpose(ps, nat[:, kc * 128:(kc + 1) * 128], ident)
            nc.scalar.copy(out=xtauT[kc][:, pt * 128:(pt + 1) * 128], in_=ps)

    # Z = |x_tau|^2 via square + ones-matmul
    zp = wrow.tile([1, BL], F32, tag="w", name="zp")
    for kc in range(2):
        sq = acts.tile([128, BL], adt, tag="sq", name=f"zsq{kc}")
        nc.scalar.square(out=sq, in_=xtauT[kc])
        nc.tensor.matmul(zp, ones_t, sq, start=(kc == 0), stop=(kc == 1))
    nc.vector.tensor_copy(out=z_row, in_=zp)

    critic(xtauT, vtau_row)

    # ---- main loop: actor MLP + squared-norm reduction over all points ----
    for g in range(NG):
        xt = []
        for kc in range(2):
            t = xin.tile([128, NB, N_T], xdt, tag="x")
            nc.sync.dma_start(out=t, in_=x_d[g, kc])
            xt.append(t)
        xr = [t.rearrange("p b t -> p (b t)") for t in xt]

        if MM_MODE == "bf16":
            xb = []
            for kc in range(2):
                c = acts.tile([128, NPTS], BF16, tag="xb", name=f"xb{kc}")
                nc.vector.tensor_copy(out=c, in_=xr[kc])
                xb.append(c)
        else:
            xb = xr

        # x0 gather (t=0 column of each sample) for the end-of-kernel critic
        for kc in range(2):
            nc.gpsimd.tensor_copy(
                out=x0b[kc][:, g * NB:(g + 1) * NB],
                in_=xt[kc][:, :, 0:1].rearrange("p b t -> p (b t)"))

        # layer 1 (relu on ACT), layer 2 (relu on DVE), layer 3 -> u^2 on ACT
        h = xb
        for layer in range(2):
            nh = []
            for m in range(2):
                ps = psum.tile([128, NPTS], F32, tag="mm", name=f"ps{layer}{m}")
                for kc in range(2):
                    nc.tensor.matmul(ps, aw[layer][:, kc, m * 128:(m + 1) * 128],
                                     h[kc], start=(kc == 0), stop=(kc == 1))
                ht = acts.tile([128, NPTS], adt, tag="h", name=f"h{layer}{m}")
                if layer == 0:
                    nc.scalar.activation(out=ht, in_=ps,
                                         func=mybir.ActivationFunctionType.Relu,
                                         bias=ab[layer][:, m:m + 1], scale=1.0)
                else:
                    nc.vector.tensor_scalar(
                        out=ht, in0=ps, scalar1=ab[layer][:, m:m + 1], scalar2=0.0,
                        op0=mybir.AluOpType.add, op1=mybir.AluOpType.max)
                nh.append(ht)
            h = nh

        wp = wrow.tile([1, NPTS], F32, tag="w", name="wp")
        for m in range(2):
            ps = psum.tile([128, NPTS], F32, tag="mm", name=f"ps3{m}")
            for kc in range(2):
                nc.tensor.matmul(ps, aw[2][:, kc, m * 128:(m + 1) * 128],
                                 h[kc], start=(kc == 0), stop=(kc == 1))
            u2 = acts.tile([128, NPTS], adt, tag="sq", name=f"u2{m}")
            nc.scalar.activation(out=u2, in_=ps,
                                 func=mybir.ActivationFunctionType.Square,
                                 bias=ab[2][:, m:m + 1], scale=1.0)
            nc.tensor.matmul(wp, ones_t, u2, start=(m == 0), stop=False)
        for kc in range(2):
            sq = acts.tile([128, NPTS], adt, tag="sq", name=f"sq{kc}")
            nc.gpsimd.tensor_mul(sq, xb[kc], xb[kc])
            nc.tensor.matmul(wp, ones_t, sq, start=False, stop=(kc == 1))

        # weighted per-sample time reduction
        wm = rows.tile([1, NPTS], F32, tag="wm")
        nc.sync.dma_start(out=wm, in_=wmat_d[g])
        ws = rows.tile([1, NPTS], F32, tag="ws")
        nc.vector.tensor_mul(ws, wp, wm)
        nc.vector.reduce_sum(out=y_buf[0:1, g * NB:(g + 1) * NB],
                             in_=ws.rearrange("p (b t) -> p b t", b=NB),
                             axis=mybir.AxisListType.X)

    critic(x0b, v0_row)

    # ---- final combine: delta = y + mask*(Z - vtau) + vtau - v0 + crow ----
    f = [rows.tile([1, BL], F32, tag="fin", name=f"fin{_i}") for _i in range(2)]
    nc.vector.tensor_sub(f[0], z_row, vtau_row)
    nc.vector.tensor_mul(f[1], mask_t, f[0])
    nc.vector.tensor_add(f[0], f[1], vtau_row)
    nc.vector.tensor_sub(f[1], f[0], v0_row)
    nc.vector.tensor_add(f[0], f[1], y_buf)
    out_t = rows.tile([1, BL], F32, tag="out")
    nc.vector.tensor_add(out_t, f[0], crow_t)
    nc.sync.dma_start(out=out_d, in_=out_t)


if os.environ.get("BASS_LDW_OPT", "0") == "1":
    # experiment: let walrus pipeline/dedup LDWEIGHTS (off by default upstream)
    import concourse.bass_utils as _BU
    _orig_run_command = _BU.run_command

    def _run_command_ldwopt(argv, **kwargs):
        argv = ["--enable-ldw-opt=true" if a == "--enable-ldw-opt=false" else a
                for a in argv]
        return _orig_run_command(argv, **kwargs)

    _BU.run_command = _run_command_ldwopt


_BUILT = None


def build():
    global _BUILT
    if _BUILT is not None:
        return _BUILT
    nc = bacc.Bacc("TRN2", target_bir_lowering=False, debug=False)
    with tile.TileContext(nc) as tc:
        with ExitStack() as ctx:
            _kernel_body(ctx, tc)
    nc.compile()
    _BUILT = nc
    return nc


def _affine(bn):
    g, b, m, v = (np.asarray(t, np.float64) for t in bn)
    a = g / np.sqrt(v + BN_EPS)
    return a, b - m * a


def fold_params(p):
    a_in, c_in = _affine(p['bn_in'])
    W1, W2 = (np.asarray(W, np.float64) for W in p['Ws'])
    (a1, c1), (a2, c2) = (_affine(bn) for bn in p['bns'])
    Wout = np.asarray(p['Wout'], np.float64)
    bout = np.asarray(p['bout'], np.float64)
    a_o, c_o = _affine(p['bn_out'])
    U1 = (a_in[:, None] * W1) * a1[None, :]
    d1 = (c_in @ W1) * a1 + c1
    U2 = W2 * a2[None, :]
    U3 = Wout * a_o[None, :]
    d3 = bout * a_o + c_o
    return [(U1, d1), (U2, c2), (U3, d3)]


def prepare_in_maps(x, x_tau, tau, ExitIndex, critic_params, actor_params):
    x = np.asarray(x, np.float32)
    # reshard + relayout: [B, 256, 65] -> per-core [NG, 2, 128, NB, N_T] blocks
    xw = x[:, :, :N_T].reshape(NCORES, NG, NB, 2, 128, N_T)
    xc = np.ascontiguousarray(xw.transpose(0, 1, 3, 4, 2, 5))
    x_tau = np.ascontiguousarray(np.asarray(x_tau, np.float32))
    tau = np.asarray(tau, np.float32)
    ei = np.asarray(ExitIndex, np.int32)

    af = fold_params(actor_params)
    cf = fold_params(critic_params)

    t = np.arange(N_T, dtype=np.int32)[None, :]
    eib = ei[:, None]
    wmat = np.where(t < eib, np.float32(DT),
                    np.where(t == eib, tau[:, None] - t.astype(np.float32) * np.float32(DT),
                             np.float32(0))).astype(np.float32)
    maskF = (ei < N_T).astype(np.float32)
    d3c = np.float32(cf[2][1][0])
    crow = (-maskF * d3c).astype(np.float32)

    shared = {
        "aw0": _mm_np(af[0][0]), "aw1": _mm_np(af[1][0]), "aw2": _mm_np(af[2][0]),
        "ab0": np.asarray(af[0][1], np.float32),
        "ab1": np.asarray(af[1][1], np.float32),
        "ab2": np.asarray(af[2][1], np.float32),
        "cw0": _mm_np(cf[0][0]), "cw1": _mm_np(cf[1][0]),
        "cb0": np.asarray(cf[0][1], np.float32),
        "cb1": np.asarray(cf[1][1], np.float32),
        "cv": _mm_np(cf[2][0][:, 0]),
        "ones": _mm_np(np.ones(128)),
    }

    in_maps = []
    for c in range(NCORES):
        s = slice(c * BL, (c + 1) * BL)
        in_maps.append({
            "x": xc[c],
            "x_tau": x_tau[s],
            "wmat": np.ascontiguousarray(wmat[s].reshape(NG, 1, NPTS)),
            "mask": maskF[s].reshape(1, BL),
            "crow": crow[s].reshape(1, BL),
            **shared,
        })
    return in_maps


def kernel(dw, x, x_tau, tau, ExitIndex, critic_params, actor_params):
    in_maps = prepare_in_maps(x, x_tau, tau, ExitIndex, critic_params, actor_params)
    nc = build()
    try:
        res = run_bass_kernel_spmd(nc, in_maps, core_ids=list(range(NCORES)))
    except Exception:
        # transient NRT_EXEC_UNIT_UNRECOVERABLE has been observed to clear on
        # the next attempt; retry once before giving up
        res = run_bass_kernel_spmd(nc, in_maps, core_ids=list(range(NCORES)))
    delta = np.concatenate([res.results[c]["delta"][0] for c in range(NCORES)])
    return delta.astype(np.float32)


if __name__ == "__main__":
    build()
    print("build ok")


# revision 37
# speedup vs baseline: 1.1195x; 1.0245x over previous
"""Trainium2 Bass kernel for nn_CriticModel (BSDE critic-delta forward).

Math (see reference):
  xt = all (sample, t<64) points of x           [B*64, 256]
  u = actor_mlp(xt); w = |xt|^2 + |u|^2         (BN folded into dense layers)
  y = sum_t w * weight(t; tau, ExitIndex)
  delta = y + where(ExitIndex<64, |x_tau|^2, critic(x_tau)) - critic(x[:,:,0])

Strategy: pure data parallel over 8 NeuronCores (512 samples each).
On-device per core: stream groups of 8 samples x 64 timesteps (512 points,
features on partitions, 2 K-chunks of 128), run the 3-layer folded MLP on
the PE, square+ones-matmul reductions for |x|^2 and |u|^2, then a
per-sample weighted time reduction. Critic runs on x_tau (PE-transposed)
and on gathered t=0 columns. BN folding, the ragged time-weight matrix and
the exit mask are precomputed on host (tiny tensors only).

Dtype modes (BASS_MM_MODE):
  f32r (default): fp32r weights + activations — fp32 bits stream through the
        PE at 1 cycle/col (N>=256) with no cast passes and ~1e-4 accuracy.
  bf16: everything bf16 (DVE cast passes for x).
  f32:  plain fp32 matmuls (4 cycles/col, slow; reference fallback).
  (mix bf16xf32r is rejected by walrus: no 32-bit x 16-bit matmuls.)
"""

import os
import sys
from contextlib import ExitStack

for _p in ("/opt/trn_rl_repo",):
    if _p not in sys.path:
        sys.path.insert(0, _p)

import numpy as np

import concourse.bass as bass
import concourse.tile as tile
from concourse import bacc, mybir
from concourse.bass_utils import run_bass_kernel_spmd
from concourse.masks import make_identity

DIM = 256
N_T = 64
DT = 1.0 / N_T
BN_EPS = 1e-6
B = 4096
NCORES = 8
BL = B // NCORES          # samples per core
NB = 8                    # samples per group (NB*N_T = 512 matmul columns)
NG = BL // NB             # groups per core
NPTS = NB * N_T           # 512
F32 = mybir.dt.float32
BF16 = mybir.dt.bfloat16
F32R = mybir.dt.float32r

MM_MODE = os.environ.get("BASS_MM_MODE", "f32r")


def _w_dt():
    """Weight (stationary operand) dtype."""
    return {"mix": BF16, "bf16": BF16, "f32r": F32R, "f32": F32}[MM_MODE]


def _a_dt():
    """Activation (moving operand) storage dtype."""
    return {"mix": F32R, "bf16": BF16, "f32r": F32R, "f32": F32}[MM_MODE]


def _mm_np(a):
    """Host-side storage format for weights."""
    if _w_dt() == BF16:
        import ml_dtypes
        return np.asarray(a, np.float32).astype(ml_dtypes.bfloat16)
    return np.asarray(a, np.float32)


def _kernel_body(ctx, tc):
    nc = tc.nc
    wdt = _w_dt()
    adt = _a_dt()

    # x arrives host-resharded as per-(group, k-chunk) contiguous blocks so
    # each DMA descriptor moves a full 2KB partition row (vs 256B strided).
    xdt = adt if MM_MODE == "f32r" else F32
    x_d = nc.dram_tensor("x", [NG, 2, 128, NB, N_T], xdt, kind="ExternalInput").ap()
    xtau_d = nc.dram_tensor("x_tau", [BL, DIM], F32, kind="ExternalInput").ap()
    wmat_d = nc.dram_tensor("wmat", [NG, 1, NPTS], F32, kind="ExternalInput").ap()
    mask_d = nc.dram_tensor("mask", [1, BL], F32, kind="ExternalInput").ap()
    crow_d = nc.dram_tensor("crow", [1, BL], F32, kind="ExternalInput").ap()
    aw_d = [nc.dram_tensor(f"aw{i}", [DIM, DIM], wdt, kind="ExternalInput").ap()
            for i in range(3)]
    ab_d = [nc.dram_tensor(f"ab{i}", [DIM], F32, kind="ExternalInput").ap()
            for i in range(3)]
    cw_d = [nc.dram_tensor(f"cw{i}", [DIM, DIM], wdt, kind="ExternalInput").ap()
            for i in range(2)]
    cb_d = [nc.dram_tensor(f"cb{i}", [DIM], F32, kind="ExternalInput").ap()
            for i in range(2)]
    cv_d = nc.dram_tensor("cv", [DIM], wdt, kind="ExternalInput").ap()
    ones_d = nc.dram_tensor("ones", [128], wdt, kind="ExternalInput").ap()
    out_d = nc.dram_tensor("delta", [1, BL], F32, kind="ExternalOutput").ap()

    consts = ctx.enter_context(tc.tile_pool(name="consts", bufs=1))
    xin = ctx.enter_context(tc.tile_pool(name="xin", bufs=10))
    acts = ctx.enter_context(tc.tile_pool(name="acts", bufs=8))
    rows = ctx.enter_context(tc.tile_pool(name="rows", bufs=6))
    persist = ctx.enter_context(tc.tile_pool(name="persist", bufs=1))
    psum = ctx.enter_context(tc.tile_pool(name="psum", bufs=5, space="PSUM"))
    wrow = ctx.enter_context(tc.tile_pool(name="wrow", bufs=3, space="PSUM"))

    # ---- constants ----
    # weight tiles: [K=128, kc, M=256]; lhsT slice = [:, kc, m*128:(m+1)*128]
    def load_w(dram, nm):
        t = consts.tile([128, 2, DIM], wdt, tag=nm, name=nm)
        for kc in range(2):
            nc.sync.dma_start(out=t[:, kc, :], in_=dram[kc * 128:(kc + 1) * 128, :])
        return t

    aw = [load_w(d, f"aw{i}_t") for i, d in enumerate(aw_d)]
    cw = [load_w(d, f"cw{i}_t") for i, d in enumerate(cw_d)]

    def load_b(dram, nm):
        t = consts.tile([128, 2], F32, tag=nm, name=nm)
        nc.sync.dma_start(out=t, in_=dram.rearrange("(m p) -> p m", p=128))
        return t

    ab = [load_b(d, f"ab{i}_t") for i, d in enumerate(ab_d)]
    cb = [load_b(d, f"cb{i}_t") for i, d in enumerate(cb_d)]
    cv = consts.tile([128, 2], wdt, tag="cv")
    nc.sync.dma_start(out=cv, in_=cv_d.rearrange("(m p) -> p m", p=128))

    ones_t = consts.tile([128, 1], wdt, tag="ones")
    nc.sync.dma_start(out=ones_t, in_=ones_d.rearrange("(p o) -> p o", o=1))
    ident = consts.tile([128, 128], F32, tag="ident")
    make_identity(nc, ident)
    mask_t = consts.tile([1, BL], F32, tag="mask")
    nc.sync.dma_start(out=mask_t, in_=mask_d)
    crow_t = consts.tile([1, BL], F32, tag="crow")
    nc.sync.dma_start(out=crow_t, in_=crow_d)

    # ---- persistent ----
    x0b = [persist.tile([128, BL], adt, tag=f"x0b{k}", name=f"x0b{k}")
           for k in range(2)]
    xtauT = [persist.tile([128, BL], adt, tag=f"xtauT{k}", name=f"xtauT{k}")
             for k in range(2)]
    y_buf = persist.tile([1, BL], F32, tag="y_buf")
    z_row = persist.tile([1, BL], F32, tag="z_row")
    vtau_row = persist.tile([1, BL], F32, tag="vtau_row")
    v0_row = persist.tile([1, BL], F32, tag="v0_row")

    def critic(chunks, out_row):
        """3-layer folded critic on [128, BL] feature-major chunks -> [1, BL]."""
        h = chunks
        for layer in range(2):
            nh = []
            for m in range(2):
                ps = psum.tile([128, BL], F32, tag="mm")
                for kc in range(2):
                    nc.tensor.matmul(ps, cw[layer][:, kc, m * 128:(m + 1) * 128],
                                     h[kc], start=(kc == 0), stop=(kc == 1))
                ht = acts.tile([128, BL], adt, tag="h")
                nc.scalar.activation(out=ht, in_=ps,
                                     func=mybir.ActivationFunctionType.Relu,
                                     bias=cb[layer][:, m:m + 1], scale=1.0)
                nh.append(ht)
            h = nh
        vp = wrow.tile([1, BL], F32, tag="w")
        for kc in range(2):
            nc.tensor.matmul(vp, cv[:, kc:kc + 1], h[kc],
                             start=(kc == 0), stop=(kc == 1))
        nc.vector.tensor_copy(out=out_row, in_=vp)

    # ---- x_tau: natural load + PE transpose into feature-major chunks ----
    for pt in range(BL // 128):
        nat = xin.tile([128, DIM], F32, tag="xtau_nat", name=f"nat{pt}")
        nc.sync.dma_start(out=nat, in_=xtau_d[pt * 128:(pt + 1) * 128, :])
        for kc in range(2):
            ps = psum.tile([128, 128], F32, tag="mm", name=f"tr{kc}")
            nc.tensor.transpose(ps, nat[:, kc * 128:(kc + 1) * 128], ident)
            nc.scalar.copy(out=xtauT[kc][:, pt * 128:(pt + 1) * 128], in_=ps)

    # Z = |x_tau|^2 via square + ones-matmul
    zp = wrow.tile([1, BL], F32, tag="w", name="zp")
    for kc in range(2):
        sq = acts.tile([128, BL], adt, tag="sq", name=f"zsq{kc}")
        nc.scalar.square(out=sq, in_=xtauT[kc])
        nc.tensor.matmul(zp, ones_t, sq, start=(kc == 0), stop=(kc == 1))
    nc.vector.tensor_copy(out=z_row, in_=zp)

    critic(xtauT, vtau_row)

    # ---- main loop: actor MLP + squared-norm reduction over all points ----
    for g in range(NG):
        xt = []
        for kc in range(2):
            t = xin.tile([128, NB, N_T], xdt, tag="x")
            nc.sync.dma_start(out=t, in_=x_d[g, kc])
            xt.append(t)
        xr = [t.rearrange("p b t -> p (b t)") for t in xt]

        if MM_MODE == "bf16":
            xb = []
            for kc in range(2):
                c = acts.tile([128, NPTS], BF16, tag="xb", name=f"xb{kc}")
                nc.vector.tensor_copy(out=c, in_=xr[kc])
                xb.append(c)
        else:
            xb = xr

        # x0 gather (t=0 column of each sample) for the end-of-kernel critic
        for kc in range(2):
            nc.gpsimd.tensor_copy(
                out=x0b[kc][:, g * NB:(g + 1) * NB],
                in_=xt[kc][:, :, 0:1].rearrange("p b t -> p (b t)"))

        # layer 1 (relu on ACT), layer 2 (relu on DVE), layer 3 -> u^2 on ACT
        h = xb
        for layer in range(2):
            nh = []
            for m in range(2):
                ps = psum.tile([128, NPTS], F32, tag="mm", name=f"ps{layer}{m}")
                for kc in range(2):
                    nc.tensor.matmul(ps, aw[layer][:, kc, m * 128:(m + 1) * 128],
                                     h[kc], start=(kc == 0), stop=(kc == 1))
                ht = acts.tile([128, NPTS], adt, tag="h", name=f"h{layer}{m}")
                if layer == 0:
                    nc.scalar.activation(out=ht, in_=ps,
                                         func=mybir.ActivationFunctionType.Relu,
                                         bias=ab[layer][:, m:m + 1], scale=1.0)
                else:
                    nc.vector.tensor_scalar(
                        out=ht, in0=ps, scalar1=ab[layer][:, m:m + 1], scalar2=0.0,
                        op0=mybir.AluOpType.add, op1=mybir.AluOpType.max)
                nh.append(ht)
            h = nh

        wp = wrow.tile([1, NPTS], F32, tag="w", name="wp")
        for m in range(2):
            ps = psum.tile([128, NPTS], F32, tag="mm", name=f"ps3{m}")
            for kc in range(2):
                nc.tensor.matmul(ps, aw[2][:, kc, m * 128:(m + 1) * 128],
                                 h[kc], start=(kc == 0), stop=(kc == 1))
            u2 = acts.tile([128, NPTS], adt, tag="sq", name=f"u2{m}")
            nc.scalar.activation(out=u2, in_=ps,
                                 func=mybir.ActivationFunctionType.Square,
                                 bias=ab[2][:, m:m + 1], scale=1.0)
            nc.tensor.matmul(wp, ones_t, u2, start=(m == 0), stop=False)
        for kc in range(2):
            sq = acts.tile([128, NPTS], adt, tag="sq", name=f"sq{kc}")
            nc.gpsimd.tensor_mul(sq, xb[kc], xb[kc])
            nc.tensor.matmul(wp, ones_t, sq, start=False, stop=(kc == 1))

        # weighted per-sample time reduction
        wm = rows.tile([1, NPTS], F32, tag="wm")
        nc.sync.dma_start(out=wm, in_=wmat_d[g])
        ws = rows.tile([1, NPTS], F32, tag="ws")
        nc.vector.tensor_mul(ws, wp, wm)
        nc.vector.reduce_sum(out=y_buf[0:1, g * NB:(g + 1) * NB],
                             in_=ws.rearrange("p (b t) -> p b t", b=NB),
                             axis=mybir.AxisListType.X)

    critic(x0b, v0_row)

    # ---- final combine: delta = y + mask*(Z - vtau) + vtau - v0 + crow ----
    f = [rows.tile([1, BL], F32, tag="fin", name=f"fin{_i}") for _i in range(2)]
    nc.vector.tensor_sub(f[0], z_row, vtau_row)
    nc.vector.tensor_mul(f[1], mask_t, f[0])
    nc.vector.tensor_add(f[0], f[1], vtau_row)
    nc.vector.tensor_sub(f[1], f[0], v0_row)
    nc.vector.tensor_add(f[0], f[1], y_buf)
    out_t = rows.tile([1, BL], F32, tag="out")
    nc.vector.tensor_add(out_t, f[0], crow_t)
    nc.sync.dma_start(out=out_d, in_=out_t)


if os.environ.get("BASS_LDW_OPT", "0") == "1":
    # experiment: let walrus pipeline/dedup LDWEIGHTS (off by default upstream)
    import concourse.bass_utils as _BU
    _orig_run_command = _BU.run_command

    def _run_command_ldwopt(argv, **kwargs):
        argv = ["--enable-ldw-opt=true" if a == "--enable-ldw-opt=false" else a
                for a in argv]
        return _orig_run_command(argv, **kwargs)

    _BU.run_command = _run_command_ldwopt


_BUILT = None


def build():
    global _BUILT
    if _BUILT is not None:
        return _BUILT
    nc = bacc.Bacc("TRN2", target_bir_lowering=False, debug=False)
    with tile.TileContext(nc) as tc:
        with ExitStack() as ctx:
            _kernel_body(ctx, tc)
    nc.compile()
    _BUILT = nc
    return nc


def _affine(bn):
    g, b, m, v = (np.asarray(t, np.float64) for t in bn)
    a = g / np.sqrt(v + BN_EPS)
    return a, b - m * a


def fold_params(p):
    a_in, c_in = _affine(p['bn_in'])
    W1, W2 = (np.asarray(W, np.float64) for W in p['Ws'])
    (a1, c1), (a2, c2) = (_affine(bn) for bn in p['bns'])
    Wout = np.asarray(p['Wout'], np.float64)
    bout = np.asarray(p['bout'], np.float64)
    a_o, c_o = _affine(p['bn_out'])
    U1 = (a_in[:, None] * W1) * a1[None, :]
    d1 = (c_in @ W1) * a1 + c1
    U2 = W2 * a2[None, :]
    U3 = Wout * a_o[None, :]
    d3 = bout * a_o + c_o
    return [(U1, d1), (U2, c2), (U3, d3)]


def prepare_in_maps(x, x_tau, tau, ExitIndex, critic_params, actor_params):
    x = np.asarray(x, np.float32)
    # reshard + relayout: [B, 256, 65] -> per-core [NG, 2, 128, NB, N_T] blocks
    xw = x[:, :, :N_T].reshape(NCORES, NG, NB, 2, 128, N_T)
    xc = np.ascontiguousarray(xw.transpose(0, 1, 3, 4, 2, 5))
    x_tau = np.ascontiguousarray(np.asarray(x_tau, np.float32))
    tau = np.asarray(tau, np.float32)
    ei = np.asarray(ExitIndex, np.int32)

    af = fold_params(actor_params)
    cf = fold_params(critic_params)

    t = np.arange(N_T, dtype=np.int32)[None, :]
    eib = ei[:, None]
    wmat = np.where(t < eib, np.float32(DT),
                    np.where(t == eib, tau[:, None] - t.astype(np.float32) * np.float32(DT),
                             np.float32(0))).astype(np.float32)
    maskF = (ei < N_T).astype(np.float32)
    d3c = np.float32(cf[2][1][0])
    crow = (-maskF * d3c).astype(np.float32)

    shared = {
        "aw0": _mm_np(af[0][0]), "aw1": _mm_np(af[1][0]), "aw2": _mm_np(af[2][0]),
        "ab0": np.asarray(af[0][1], np.float32),
        "ab1": np.asarray(af[1][1], np.float32),
        "ab2": np.asarray(af[2][1], np.float32),
        "cw0": _mm_np(cf[0][0]), "cw1": _mm_np(cf[1][0]),
        "cb0": np.asarray(cf[0][1], np.float32),
        "cb1": np.asarray(cf[1][1], np.float32),
        "cv": _mm_np(cf[2][0][:, 0]),
        "ones": _mm_np(np.ones(128)),
    }

    in_maps = []
    for c in range(NCORES):
        s = slice(c * BL, (c + 1) * BL)
        in_maps.append({
            "x": xc[c],
            "x_tau": x_tau[s],
            "wmat": np.ascontiguousarray(wmat[s].reshape(NG, 1, NPTS)),
            "mask": maskF[s].reshape(1, BL),
            "crow": crow[s].reshape(1, BL),
            **shared,
        })
    return in_maps


def kernel(dw, x, x_tau, tau, ExitIndex, critic_params, actor_params):
    in_maps = prepare_in_maps(x, x_tau, tau, ExitIndex, critic_params, actor_params)
    nc = build()
    try:
        res = run_bass_kernel_spmd(nc, in_maps, core_ids=list(range(NCORES)))
    except Exception:
        # transient NRT_EXEC_UNIT_UNRECOVERABLE has been observed to clear on
        # the next attempt; retry once before giving up
        res = run_bass_kernel_spmd(nc, in_maps, core_ids=list(range(NCORES)))
    delta = np.concatenate([res.results[c]["delta"][0] for c in range(NCORES)])
    return delta.astype(np.float32)


if __name__ == "__main__":
    build()
    print("build ok")
